# revision 1
# baseline (speedup 1.0000x reference)
"""Trainium2 Bass kernel for nn_Conv2dBN_fake_int8.

Math: the reference quantizes x and weight to int8 levels, then computes
out[b,l,o] = sum_k lut[qf[b,l,k]+128, qw[o,k]+128] with lut the exact
product table lut[i,j] = (i-128)*(j-128), so the LUT-GEMM is an integer
GEMM == a 3x3 pad-1 conv on the quantized values.  We verify the product
property of the passed lut on the host (cheap) and run the conv on the
TensorEngine in bf16 (all products/partial sums are integers < 2^24, so
fp32 PSUM accumulation is exact).  Dequant + fake-quant of the activation
runs on the vector/scalar engines; round-half-even is implemented with the
+/- 1.5*2^23 magic-number trick so it matches jnp.round.

Sharding: data-parallel over batch B=8 across the 8 NeuronCores (one image
per core); weights/scales replicated.

Performance structure (per core):
- x is DMA'd with a partition-broadcast AP into 128 partitions (two copies
  of the 64-channel image), so the 9 conv taps can run as 4 concurrent
  row-group *pairs* on the PE array (tap t in rows 0-63, tap t+1 in rows
  64-127 via tile_position) plus one paired leftover tap - halving
  TensorEngine streaming time vs 9 serial K=64 matmuls.
- w|sw|s2|b2 are packed into one DRAM tensor -> one DMA -> one semaphore
  (this walrus build allows only ONE sync wait per compute instruction, so
  every instruction is arranged to have at most one uncovered producer).
"""

import numpy as np

# Problem shape (hardcoded; harness runs kernel.py standalone).
B, C, H, W = 8, 64, 32, 32
O, KH, KW = 64, 3, 3
OH, OW = 32, 32
L = OH * OW          # 1024
NT = KH * KW         # 9 taps
K = C * NT           # 576
PADW = W + 2         # 34
PADA = (H + 2) * PADW  # 1156
NCORES = 8
CHUNK = 512          # fp32 free elements per PSUM bank
RPC = CHUNK // OW    # output rows per PSUM chunk (16)
MAGIC = 12582912.0   # 1.5*2^23 -> fp32 round-to-nearest-even via add/sub
# Weight columns are packed on the host in transpose-pair-block order:
# blocks (0,3),(1,4),(2,5),(6,7),(8,8) -> tap sequence [0,3,1,4,2,5,6,7,8,8],
# each block 2 taps x 64 channels = 128 contiguous columns, so one PE
# transpose per block produces the stacked [tap*64+c, o] lhsT layout.
TAP_ORDER = [0, 3, 1, 4, 2, 5, 6, 7, 8, 8]
KP = len(TAP_ORDER) * C  # 640 packed weight columns
WSB_COLS = KP + 3 + O // 2  # [w-pairs | isw | s2 | b2 | bf16-identity]

_nc_cache = {}


def _make_tc_class():
    """TileContext whose kernel-tail drain is split into a chain of
    single-wait Drain instructions: the walrus build used here allows only
    one sync-wait command per instruction, while stock Tile emits one drain
    waiting on every processor at once.  Sequentially waiting on the same
    set of semaphores is synchronization-equivalent."""
    import concourse.tile as tile
    from concourse import mybir
    from concourse.vector_clock import ScopedClock

    class SingleWaitDrainTC(tile.TileContext):
        def _drain_and_barrier(self, tick_clock, wait_clock):
            drain_inst = self.nc.sync.drain()
            wait_clock.add_sem_waits(
                drain_inst.ins, ScopedClock({None: tick_clock.global_clock})
            )
            si = drain_inst.ins.sync_info
            if si is not None and len(si.on_wait) > 1:
                waits = list(si.on_wait)
                updates = list(si.on_update)
                drain_inst.ins.sync_info = mybir.SyncInfo(
                    on_wait=waits[:1], on_update=[]
                )
                for i, w in enumerate(waits[1:]):
                    d = self.nc.sync.drain()
                    last = i == len(waits) - 2
                    d.ins.sync_info = mybir.SyncInfo(
                        on_wait=[w], on_update=updates if last else []
                    )
            self.nc.all_engine_barrier()
            assert self.sems is not None
            popped = self.nc._tile_sem_poison_stack.pop()
            assert popped is self._sem_poison
            self.nc.clear_and_free_semaphores(list(self.sems.allocated().values()))
            self.nc.all_engine_barrier()

    return SingleWaitDrainTC


def _build(sf: float, sa: float):
    import concourse.bass as bass
    import concourse.tile as tile
    from concourse import mybir

    dt = mybir.dt
    alu = mybir.AluOpType
    act = mybir.ActivationFunctionType

    nc = bass.Bass(
        "TRN2",
        debug=False,
        enable_asserts=False,
        target_bir_lowering=False,
        num_devices=NCORES,
    )

    x_d = nc.dram_tensor("x", [C, L], dt.float32, kind="ExternalInput").ap()
    wsb_d = nc.dram_tensor("wsb", [O, WSB_COLS], dt.float32, kind="ExternalInput").ap()
    out_d = nc.dram_tensor("out", [O, L], dt.float32, kind="ExternalOutput").ap()

    with _make_tc_class()(nc) as tc:
        with (
            tc.tile_pool(name="per", bufs=1) as per,
            tc.tile_pool(name="dq", bufs=2) as dq,
            tc.tile_pool(name="ps_acc", bufs=2, space="PSUM") as ps_acc,
            tc.tile_pool(name="ps_t", bufs=1, space="PSUM") as ps_t,
        ):
            # ---------------- loads ----------------
            # x split into two DMAs on separate queues (halves the transfer
            # tail that gates the quantize pipeline); issued from the ACT
            # engine, which clears the boot barrier ~1.5us before SP.
            x2 = per.tile([C, L], dt.float32)
            nc.scalar.dma_start(out=x2[:, 0 : L // 2], in_=x_d[:, 0 : L // 2])
            nc.scalar.dma_start(out=x2[:, L // 2 : L], in_=x_d[:, L // 2 : L])

            wsb = per.tile([O, WSB_COLS], dt.float32)
            nc.sync.dma_start(out=wsb, in_=wsb_d)
            w_sb = wsb[:, 0:KP]
            isw_sb = wsb[:, KP : KP + 1]       # 1/weight-quant-scale
            s2_sb = wsb[:, KP + 1 : KP + 2]    # sf*sw/sa
            b2_sb = wsb[:, KP + 2 : KP + 3]    # bias/sa
            # bf16 identity matrix shipped from the host inside wsb
            ident = wsb[:, KP + 3 : WSB_COLS].bitcast(dt.bfloat16)

            # early ACT touch of wsb so the dequant Activations only need a
            # single (PE) wait later - covers the wsb DMA queue on ACT.
            act_cover = per.tile([O, 1], dt.float32)
            nc.scalar.mul(act_cover, s2_sb, 1.0)
            # same for DVE: stage s2|b2 through an early DVE copy so the DVE
            # dequant only has the PE wait.
            dve_sb = per.tile([O, 2], dt.float32)
            nc.vector.tensor_copy(out=dve_sb, in_=wsb[:, KP + 1 : KP + 3])

            # ------- quantize x -> bf16 into zero-padded tile A lower -------
            # qf = clip(round_half_even(x * (1/sf)), -128, 127)
            # pipelined in two pixel-halves across ACT -> GPSIMD -> DVE.
            qxa = per.tile([2 * C, PADA], dt.bfloat16)
            qxb = per.tile([2 * C, PADA], dt.bfloat16)
            # only tile A's pad border needs zeroing (everything else read is
            # written); three small strided memsets instead of a full clear.
            qa3 = qxa.rearrange("c (r col) -> c r col", col=PADW)
            qb3 = qxb.rearrange("c (r col) -> c r col", col=PADW)
            nc.vector.memset(qxa[0:C, 0:PADW], 0.0)              # pad row 0
            # right pad col of row r + left pad col of row r+1, rows 0..32
            side_pads = bass.AP(
                tensor=qxa.tensor, offset=qxa.offset + W + 1,
                ap=[qxa.ap[0], [PADW, H + 1], [1, 2]],
            )
            nc.vector.memset(side_pads, 0.0)
            nc.vector.memset(qxa[0:C, PADA - PADW : PADA], 0.0)  # pad row 33

            t1 = per.tile([C, L], dt.float32)
            t2 = per.tile([C, L], dt.float32)
            inv_sf = float(np.float32(1.0) / np.float32(sf))
            HALF = L // 2
            ROWH = H // 2
            i3_insts = []
            for h in range(2):
                px = slice(h * HALF, (h + 1) * HALF)
                nc.scalar.activation(
                    out=t1[:, px], in_=x2[:, px], func=act.Copy,
                    scale=inv_sf, bias=MAGIC,
                )
                nc.vector.tensor_scalar(
                    out=t2[:, px], in0=t1[:, px], scalar1=MAGIC, scalar2=-128.0,
                    op0=alu.subtract, op1=alu.max,
                )
                i3_insts.append(nc.vector.tensor_scalar(
                    out=qa3[0:C, 1 + h * ROWH : 1 + (h + 1) * ROWH, 1 : W + 1],
                    in0=t2[:, px].rearrange("c (r col) -> c r col", col=W),
                    scalar1=127.0, scalar2=None, op0=alu.min,
                ))

            # shifted duplicates:
            #   A upper = qx << one padded row  (pairs (kh,kw)<->(kh+1,kw))
            #   B lower = qx, B upper = qx << 1 (pairs (2,0)<->(2,1))
            nc.sync.dma_start(
                out=qxa[C : 2 * C, 0 : PADA - PADW], in_=qxa[0:C, PADW:PADA]
            )
            qxb_cp = nc.vector.tensor_copy(out=qxb[0:C, :], in_=qxa[0:C, :])
            nc.sync.dma_start(
                out=qxb[C : 2 * C, 0 : PADA - 1], in_=qxa[0:C, 1:PADA]
            )

            # ------- quantize w -> bf16 [O, KP] -------
            j1 = per.tile([O, KP], dt.float32)
            nc.vector.tensor_scalar(
                out=j1, in0=w_sb, scalar1=isw_sb, scalar2=MAGIC,
                op0=alu.mult, op1=alu.add,
            )
            j2 = per.tile([O, KP], dt.float32)
            nc.vector.tensor_scalar(
                out=j2, in0=j1, scalar1=MAGIC, scalar2=-128.0,
                op0=alu.subtract, op1=alu.max,
            )
            qw = per.tile([O, KP], dt.bfloat16)
            nc.vector.tensor_scalar(
                out=qw, in0=j2, scalar1=127.0, scalar2=None, op0=alu.min,
            )

            # ------- transpose weights into stacked-pair lhsT blocks -------
            # Host packed the weight columns per block, so each block is a
            # contiguous [64, 128] slice; one PE transpose per block yields
            # [tap*64+c, o].
            #   blocks 0..2: taps (kw, kw+3)  -> rhs tile A
            #   block 3:     taps (6, 7)      -> rhs tile B
            #   block 4:     tap 8 twice (upper half unused by its matmul)
            NBLK = 5
            pst = ps_t.tile([2 * C, NBLK * O], dt.bfloat16)  # one PSUM bank
            for p in range(NBLK):
                nc.tensor.transpose(
                    pst[:, p * O : (p + 1) * O],
                    qw[:, p * 2 * C : (p + 1) * 2 * C],
                    ident,
                )
            wT = per.tile([2 * C, NBLK * O], dt.bfloat16)
            wt_cp = nc.vector.tensor_copy(out=wT, in_=pst)
            # Order the wT copy after every DVE write the matmuls read, so
            # each matmul's Ldweights DVE-wait transitively covers them and
            # the matmul itself needs at most one (DMA-queue) wait.
            from concourse.tile import add_dep_helper
            for dep in i3_insts + [qxb_cp]:
                add_dep_helper(
                    wt_cp.ins, dep.ins, sync=False,
                    reason="schedule wT copy after x-path DVE writes",
                )

            # ------- conv: K=128 stacked-pair matmuls, all at base 0 -------
            acc0 = ps_acc.tile([O, CHUNK], dt.float32, tag="acc0")
            acc1 = ps_acc.tile([O, CHUNK], dt.float32, tag="acc1")
            accs = [acc0, acc1]
            for n in range(L // CHUNK):
                r0 = n * RPC
                acc = accs[n]
                # tap 8 first: depends only on tile A lower half, so the PE
                # can start while the shifted-copy DMAs are still in flight
                nc.tensor.matmul(
                    acc, wT[0:C, 4 * O : 5 * O],
                    qa3[0:C, 2 + r0 : 2 + r0 + RPC, 2 : 2 + OW],
                    start=True, stop=False,
                )
                for kw in range(KW):
                    nc.tensor.matmul(
                        acc, wT[:, kw * O : (kw + 1) * O],
                        qa3[:, r0 : r0 + RPC, kw : kw + OW],
                        start=False, stop=False,
                    )
                nc.tensor.matmul(
                    acc, wT[:, 3 * O : 4 * O],
                    qb3[:, 2 + r0 : 2 + r0 + RPC, 0:OW],
                    start=False, stop=True,
                )

            # ------- dequant + fake-quant (per chunk, DVE/ACT mix) -------
            # ref: y = acc*sf*sw + bias; y = round(y/sa); clip; y*sa
            for n in range(L // CHUNK):
                sl = slice(n * CHUNK, (n + 1) * CHUNK)
                d1 = dq.tile([O, CHUNK], dt.float32, tag="d1")
                if n == 0:
                    # ACT path (wsb covered by act_cover; single PE wait)
                    nc.scalar.activation(
                        out=d1, in_=accs[n], func=act.Identity,
                        scale=s2_sb, bias=b2_sb,
                    )
                else:
                    nc.vector.tensor_scalar(
                        out=d1, in0=accs[n],
                        scalar1=dve_sb[:, 0:1], scalar2=dve_sb[:, 1:2],
                        op0=alu.mult, op1=alu.add,
                    )
                d2 = dq.tile([O, CHUNK], dt.float32, tag="d2")
                nc.vector.tensor_scalar(
                    out=d2, in0=d1, scalar1=MAGIC, scalar2=MAGIC,
                    op0=alu.add, op1=alu.subtract,
                )
                d3 = dq.tile([O, CHUNK], dt.float32, tag="d3")
                nc.vector.tensor_scalar(
                    out=d3, in0=d2, scalar1=-128.0, scalar2=127.0,
                    op0=alu.max, op1=alu.min,
                )
                d4 = dq.tile([O, CHUNK], dt.float32, tag="d4")
                nc.scalar.activation(
                    out=d4, in_=d3, func=act.Copy, scale=float(sa), bias=0.0
                )
                nc.sync.dma_start(out=out_d[:, sl], in_=d4)

    return nc


def _get_nc(scale_feature, scale_activation):
    sf = float(np.float32(scale_feature))
    sa = float(np.float32(scale_activation))
    key = (sf, sa)
    if key not in _nc_cache:
        _nc_cache[key] = _build(sf, sa)
    return _nc_cache[key]


def _make_in_maps(x, weight, scale_weight, bias, scale_feature, scale_activation):
    sf = np.float32(scale_feature)
    sa = np.float32(scale_activation)
    sw = scale_weight.reshape(O).astype(np.float32)
    b = bias.reshape(O).astype(np.float32)
    s2 = (sf * sw) / sa                      # fp32 per-channel dequant scale
    b2 = b / sa                              # fp32 bias in activation-steps
    import ml_dtypes

    ident16 = np.eye(O, dtype=ml_dtypes.bfloat16).view(np.uint16)
    ident_packed = (
        ident16[:, 0::2].astype(np.uint32)
        | (ident16[:, 1::2].astype(np.uint32) << 16)
    ).view(np.float32)                       # [O, O//2] fp32 carrying bf16 bits
    isw = np.float32(1.0) / sw               # fp32 reciprocal weight scale
    wr = weight.reshape(O, C, NT).astype(np.float32)
    w_pairs = np.concatenate([wr[:, :, t] for t in TAP_ORDER], axis=1)  # [O, KP]
    wsb = np.concatenate(
        [w_pairs, isw[:, None], s2[:, None], b2[:, None], ident_packed],
        axis=1,
    )
    wsb = np.ascontiguousarray(wsb, dtype=np.float32)
    return [
        {
            "x": np.ascontiguousarray(x[bb].reshape(C, L), dtype=np.float32),
            "wsb": wsb,
        }
        for bb in range(B)
    ]


def _kernel_device(x, weight, scale_feature, scale_weight, scale_activation, bias):
    from concourse import bass_utils

    nc = _get_nc(scale_feature, scale_activation)
    in_maps = _make_in_maps(
        x, weight, scale_weight, bias, scale_feature, scale_activation
    )
    res = bass_utils.run_bass_kernel_spmd(nc, in_maps, core_ids=list(range(NCORES)))
    return np.stack([r["out"].reshape(O, OH, OW) for r in res.results]).astype(
        np.float32
    )


def _kernel_numpy_lut(x, weight, lut, sf, sw, sa, bias):
    """Honest LUT-GEMM fallback (only if lut is not the product table)."""
    qf = np.clip(np.round(x / np.float32(sf)), -128.0, 127.0)
    qw = np.clip(np.round(weight / sw[:, None, None, None]), -128.0, 127.0)
    idx_w = qw.reshape(O, K).astype(np.int64) + 128
    qfp = np.pad(qf, ((0, 0), (0, 0), (1, 1), (1, 1)))
    acc = np.zeros((B, L, O), np.int64)
    for t in range(NT):
        kh, kw = divmod(t, KW)
        win = qfp[:, :, kh : kh + OH, kw : kw + OW].reshape(B, C, L)
        idx_f = win.astype(np.int64) + 128  # [B, C, L]
        for c in range(C):
            acc += lut[idx_f[:, c, :, None], idx_w[None, None, :, c * NT + t]]
    out = acc.astype(np.float32).transpose(0, 2, 1).reshape(B, O, OH, OW)
    out = out * np.float32(sf) * sw[None, :, None, None]
    out = out + bias[None, :, None, None]
    out = np.round(out / np.float32(sa))
    out = np.clip(out, -128.0, 127.0)
    return (out * np.float32(sa)).astype(np.float32)


def kernel(x, weight, lut, scale_feature, scale_weight, scale_activation, bias):
    x = np.asarray(x, dtype=np.float32)
    weight = np.asarray(weight, dtype=np.float32)
    lut = np.asarray(lut)
    scale_weight = np.asarray(scale_weight, dtype=np.float32)
    bias = np.asarray(bias, dtype=np.float32)

    i = np.arange(256, dtype=np.int64) - 128
    product = i[:, None] * i[None, :]
    if not np.array_equal(np.asarray(lut, dtype=np.int64), product):
        return _kernel_numpy_lut(
            x, weight, np.asarray(lut, dtype=np.int64),
            float(np.float32(scale_feature)), scale_weight,
            float(np.float32(scale_activation)), bias,
        )

    return _kernel_device(
        x, weight, scale_feature, scale_weight, scale_activation, bias
    )

